# revision 1
# baseline (speedup 1.0000x reference)
"""AMIPRouter Trainium2 kernel (8 NeuronCores, SPMD, no collectives).

Math restructure (exactly equivalent to the reference):
  eo[t,k,:]   = gelu(h[t] @ W1_k + b1_k) @ W2_k + b2_k
  win[s,k,:]  = sum_{t in window(s), t unmasked} eo[t,k,:]
  out[s]      = LN( sum_k w[s,k] * win[s,k,:] / cnt[s] )  at s masked & cnt>0

W2 is linear, so the windowed neighbor-sum commutes with it:
  win[s,k,:] = (sum_{t in win(s)} ghid[t,k,:]) @ W2_k + cnt[s] * b2_k
with ghid = gelu(layer1) over *unmasked* tokens only. The positional windowed
sum becomes a matmul against a host-built 0/1 selection matrix Wsel[j, m]
(j: unmasked tokens in the shard's halo range, m: masked+valid outputs). The
routing softmax w, the b2 @ w mix, and the final LayerNorm all run on the
host (pure pre/post-processing of kernel inputs/outputs); the device runs
only the three big matmul stages:
  L1 (transposed):  ghidT[f, j; k] = gelu(W1_k.T @ hg + b1_k), then PE
                    transposes back to ghid[j, f] tiles
  WIN:              A^T[f, m; k] = (ghid_k.T @ Wsel) * wbc_k, where wbc is
                    the host-computed w[m,k]/cnt[m] row broadcast over
                    partitions via a ones-matmul (the broadcast multiply
                    rides the PSUM->SBUF copy)
  L2 (transposed):  mixedT[d-chunk, m] = sum_c W2[c-chunk, d].T @ A^T[c, m],
                    streaming W2 in 1 MiB columns; each d-chunk DMAs straight
                    out after its PSUM copy.

The problem sits on the HBM ridge: all 8 cores pull the same 16 MiB W1 +
16 MiB W2 streams concurrently, so the schedule keeps one long DMA wait up
front and then paces layer 1 at the W1 arrival rate — stuttered streaming
loses more to PE clock-ramp resets (1.2 GHz for 3 us after any idle gap)
than it gains. Sharding: the flattened (batch, seq) axis is cut into 8
contiguous ranges by a minimax search balancing the padded unmasked
(halo-extended) and masked token counts against the PE cycle model; shards
may span the batch boundary (the selection matrix enforces same-batch
windows). Inputs are laid out partition-major so every DMA is linear;
compute is bf16 with f32 PSUM accumulation.
"""

import numpy as np
import ml_dtypes

BF16 = ml_dtypes.bfloat16

_B, _S, _D, _K, _F = 2, 2048, 2048, 8, 512
_NCORES = 8

_GRAPH_CACHE = {}


def _ceil_mult(x, m):
    return max(m, ((x + m - 1) // m) * m)


def _build_graph(NU, SM, SMA, NUA):
    """Build + compile the per-core Bass graph for padded sizes (NU, SM)."""
    import concourse.mybir as mybir
    from concourse import bacc
    from concourse.tile import TileContext
    from concourse.masks import make_identity
    from contextlib import ExitStack

    D, K, F = _D, _K, _F
    DC = D // 128          # 16 contract chunks of d
    FM = F // 128          # 4 f-chunks per expert
    KF = K * F // 128      # 32 contract chunks of layer 2
    JC = NU // 128
    f32 = mybir.dt.float32
    bf16 = mybir.dt.bfloat16
    AF = mybir.ActivationFunctionType

    nc = bacc.Bacc("TRN2", target_bir_lowering=False, debug=False, num_devices=_NCORES)

    # all big inputs are pre-laid-out partition-major: [128, ...]
    hgT_e = nc.declare_dram_parameter("hgT", [128, DC, NUA], bf16, isOutput=False)
    wsel_e = nc.declare_dram_parameter("wsel", [128, JC, SMA], bf16, isOutput=False)
    wi_e = nc.declare_dram_parameter("wi", [1, K, SMA], bf16, isOutput=False)
    # w1 is fm-major per expert so psum fm feeds as soon as its chunk lands
    w1_e = nc.declare_dram_parameter("w1", [K, 128, FM, DC, 128], bf16, isOutput=False)
    w2_e = nc.declare_dram_parameter("w2", [DC, 128, KF, 128], bf16, isOutput=False)
    b1_e = nc.declare_dram_parameter("b1", [128, K, FM], bf16, isOutput=False)
    out_e = nc.declare_dram_parameter("out", [DC, 128, SMA], bf16, isOutput=True)

    with TileContext(nc) as tc, ExitStack() as ctx:
        const = ctx.enter_context(tc.tile_pool(name="const", bufs=1))
        A_pool = ctx.enter_context(tc.tile_pool(name="Apool", bufs=1))
        w1p = ctx.enter_context(tc.tile_pool(name="w1p", bufs=4))
        ghp = ctx.enter_context(tc.tile_pool(name="ghp", bufs=2))
        w2sp = ctx.enter_context(tc.tile_pool(name="w2sp", bufs=6))

        # ---- DMA issuance, in priority order ----
        # hgT halves and w1[0] fm-chunks interleaved across sync+scalar so
        # pq(fm) closes as soon as its 512 KiB chunk lands
        hgT_sb = const.tile([128, DC, NUA], bf16, name="hgT_sb")
        HD = DC // 2
        nc.sync.dma_start(out=hgT_sb[:, 0:HD, :], in_=hgT_e[:][:, 0:HD, :])
        nc.scalar.dma_start(out=hgT_sb[:, HD:DC, :], in_=hgT_e[:][:, HD:DC, :])
        w1t = {}
        w1t[0] = w1p.tile([128, FM, DC, 128], bf16, name="w1t_0", tag="w1t")
        for fm in range(FM):
            eng = nc.sync if fm % 2 == 0 else nc.scalar
            eng.dma_start(out=w1t[0][:, fm], in_=w1_e[0][:, fm])
        # small shared inputs on gpsimd (slow path, fine for background)
        b1_sb = const.tile([128, K, FM], bf16, name="b1_sb")
        nc.gpsimd.dma_start(out=b1_sb, in_=b1_e[:])
        wi_row = const.tile([1, K, SMA], bf16, name="wi_row")
        nc.gpsimd.dma_start(out=wi_row, in_=wi_e[:])
        wsel_sb = const.tile([128, JC, SMA], bf16, name="wsel_sb")
        nc.gpsimd.dma_start(out=wsel_sb, in_=wsel_e[:])
        # early prefetch of experts 1 and 2, one queue each
        for kk in (1, 2):
            w1t[kk] = w1p.tile([128, FM, DC, 128], bf16, name=f"w1t_{kk}",
                               tag="w1t")
            eng = nc.sync if kk == 1 else nc.scalar
            for fm in range(0, FM, 2):
                eng.dma_start(out=w1t[kk][:, fm : fm + 2],
                              in_=w1_e[kk][:, fm : fm + 2])

        # ---- constants ----
        ident_bf = const.tile([128, 128], bf16, name="ident_bf")
        make_identity(nc, ident_bf)
        ones_bf = const.tile([1, 128], bf16, name="ones_bf")
        nc.vector.memset(ones_bf, 1.0)
        wbc = const.tile([128, K, SMA], bf16, name="wbc")

        A_tiles = {}
        w2s = {}

        def fetch_w2(dc):
            w2s[dc] = w2sp.tile([128, KF, 128], bf16, name=f"w2s_{dc}",
                                tag="w2s")
            eng = nc.sync if dc % 2 == 0 else nc.scalar
            eng.dma_start(out=w2s[dc], in_=w2_e[dc])

        # broadcast wi rows over 128 partitions via ones-matmuls; runs
        # during the initial DMA wait (wi_row lands early, it is tiny)
        with tc.tile_pool(name="psb", bufs=2, space="PSUM") as psb:
            for k in range(K):
                pb = psb.tile([128, SMA], f32, name=f"pb_{k}", tag="pb")
                nc.tensor.matmul(pb, lhsT=ones_bf[0:1, :],
                                 rhs=wi_row[0:1, k, :],
                                 start=True, stop=True)
                eng = nc.scalar if k % 2 == 0 else nc.vector
                if eng is nc.scalar:
                    nc.scalar.copy(wbc[:, k, :], pb)
                else:
                    nc.vector.tensor_copy(wbc[:, k, :], pb)

        with (
            tc.tile_pool(name="ps1", bufs=4, space="PSUM") as ps1,
            tc.tile_pool(name="psw", bufs=2, space="PSUM") as psw,
            tc.tile_pool(name="ps1t", bufs=1, space="PSUM") as ps1t,
        ):
            def layer1(k):
                # transposed: psum [f-chunk, tokens]; bias rides the gelu
                ghid_k = [
                    ghp.tile([128, F], bf16, name=f"gh_{k}_{jc}", tag=f"gh_{jc}")
                    for jc in range(JC)
                ]
                for fm in range(FM):
                    pq = ps1.tile([128, NUA], f32, name=f"pq_{k}_{fm}",
                                  tag="pg")
                    for dc in range(DC):
                        nc.tensor.matmul(
                            pq,
                            lhsT=w1t[k][:, fm, dc, :],
                            rhs=hgT_sb[:, dc, :],
                            start=(dc == 0),
                            stop=(dc == DC - 1),
                        )
                    ghT = ghp.tile([128, NUA], bf16, name=f"ghT_{k}_{fm}",
                                   tag="ghT", bufs=3)
                    nc.scalar.activation(ghT, pq, AF.Gelu,
                                         bias=b1_sb[:, k, fm : fm + 1])
                    for jc in range(JC):
                        w = min(128, NUA - jc * 128)
                        if w <= 0:
                            continue
                        pt2 = ps1t.tile([128, 128], bf16, name=f"pt2_{k}_{fm}_{jc}",
                                        tag="pt2")
                        nc.tensor.transpose(
                            pt2[0:w, :], ghT[:, jc * 128 : jc * 128 + w], ident_bf
                        )
                        eng = nc.scalar if (fm + jc) % 2 == 0 else nc.vector
                        if eng is nc.scalar:
                            nc.scalar.copy(
                                ghid_k[jc][0:w, fm * 128 : (fm + 1) * 128],
                                pt2[0:w, :],
                            )
                        else:
                            nc.vector.tensor_copy(
                                ghid_k[jc][0:w, fm * 128 : (fm + 1) * 128],
                                pt2[0:w, :],
                            )
                return ghid_k

            def window(k, ghid_k):
                for fm in range(FM):
                    At = A_pool.tile([128, SMA], bf16, name=f"A_{k}_{fm}",
                                     tag=f"A_{k}_{fm}")
                    A_tiles[(k, fm)] = At
                    for n0 in range(0, SMA, 512):
                        n1 = min(SMA, n0 + 512)
                        pw = psw.tile([128, n1 - n0], f32,
                                      name=f"pw_{k}_{fm}_{n0}", tag="pw")
                        for jc in range(JC):
                            w = min(128, NUA - jc * 128)
                            nc.tensor.matmul(
                                pw,
                                lhsT=ghid_k[jc][0:w, fm * 128 : (fm + 1) * 128],
                                rhs=wsel_sb[0:w, jc, n0:n1],
                                start=(jc == 0),
                                stop=(jc == JC - 1),
                            )
                        nc.vector.tensor_mul(At[:, n0:n1], pw,
                                             wbc[:, k, n0:n1])

            for k in range(K):
                if 2 <= k < K - 1:
                    # sync queue only: a dma_start that waits on w1 buffer
                    # reuse would head-of-line block any engine that also
                    # runs compute (scalar runs the gelu/copies)
                    w1t[k + 1] = w1p.tile([128, FM, DC, 128], bf16,
                                          name=f"w1t_{k + 1}", tag="w1t")
                    for fm in range(0, FM, 2):
                        nc.sync.dma_start(out=w1t[k + 1][:, fm : fm + 2],
                                          in_=w1_e[k + 1][:, fm : fm + 2])
                if 4 <= k <= 6:
                    # W2 prefetch trickles in on both queues from mid-run
                    fetch_w2(2 * (k - 4))
                    fetch_w2(2 * (k - 4) + 1)
                ghid_k = layer1(k)
                window(k, ghid_k)

        # ---- Phase C: transposed layer-2, streaming W2 in 1 MiB columns;
        # each d-chunk is copied out of PSUM as bf16 and DMA'd to DRAM
        # immediately (the host applies b2@w and the final LayerNorm) ----
        with (
            tc.tile_pool(name="mtp", bufs=3) as mtp,
            tc.tile_pool(name="ps2", bufs=4, space="PSUM") as ps2,
        ):
            for dc in range(DC):
                if dc + 6 < DC:
                    fetch_w2(dc + 6)
                p2 = ps2.tile([128, SMA], f32, name=f"p2_{dc}", tag="p2")
                for c in range(KF):
                    nc.tensor.matmul(
                        p2,
                        lhsT=w2s[dc][:, c, :],
                        rhs=A_tiles[(c // FM, c % FM)],
                        start=(c == 0),
                        stop=(c == KF - 1),
                    )
                mt = mtp.tile([128, SMA], bf16, name=f"mt_{dc}", tag="mt")
                nc.scalar.copy(mt, p2)
                nc.gpsimd.dma_start(out=out_e[dc], in_=mt)

    nc.compile()
    return nc


def _balance_shards(unm, valid, R):
    """Minimax search: cut the flattened (b, s) axis into 8 contiguous ranges
    minimizing the PE cycle model over (maxU, maxM), where U counts
    halo-extended unmasked tokens and M counts valid masked outputs."""
    B, S = unm.shape
    NT = B * S
    cs = np.concatenate([np.zeros((B, 1)), np.cumsum(unm, axis=1)], axis=1)
    vf = valid.reshape(-1).astype(np.int64)
    cv = np.concatenate([[0], np.cumsum(vf)])

    def ucount(p0, p1):
        tot = 0
        for b in range(B):
            lo_b, hi_b = max(p0, b * S), min(p1, (b + 1) * S)
            if lo_b >= hi_b:
                continue
            s0, s1 = lo_b - b * S, hi_b - b * S
            h0, h1 = max(0, s0 - R), min(S, s1 + R)
            tot += cs[b, h1] - cs[b, h0]
        return int(tot)

    def greedy(U, M):
        p0 = 0
        cuts = [0]
        for _ in range(_NCORES):
            lo_, hi_ = p0, NT
            while lo_ < hi_:
                mid = (lo_ + hi_ + 1) // 2
                if ucount(p0, mid) <= U and cv[mid] - cv[p0] <= M:
                    lo_ = mid
                else:
                    hi_ = mid - 1
            if lo_ == p0 and p0 < NT:
                return None
            p0 = lo_
            cuts.append(p0)
            if p0 == NT:
                break
        if p0 != NT:
            return None
        while len(cuts) < _NCORES + 1:
            cuts.append(NT)
        return cuts

    def cost_model(NUA, SMA):
        # PE column-cycles: L1 + transposes + WIN + L2
        JC = (NUA + 127) // 128
        return (512 * NUA + _K * 4 * JC * 128 // 2
                + _K * 4 * JC * SMA + 512 * SMA)

    total_u = int(unm.sum())
    total_m = int(vf.sum())
    base_u = (total_u + 2 * R * _NCORES) // _NCORES
    best = None
    for U in range(max(1, total_u // _NCORES), base_u + 64, 2):
        loM, hiM = max(1, total_m // _NCORES), total_m
        while loM < hiM:
            mid = (loM + hiM) // 2
            if greedy(U, mid):
                hiM = mid
            else:
                loM = mid + 1
        cuts = greedy(U, loM)
        if cuts is None:
            continue
        maxu = max(ucount(cuts[q], cuts[q + 1]) for q in range(_NCORES))
        maxm = max(cv[cuts[q + 1]] - cv[cuts[q]] for q in range(_NCORES))
        cost = cost_model(_ceil_mult(maxu, 4), _ceil_mult(maxm, 4))
        if best is None or cost < best[0]:
            best = (cost, cuts, maxu, maxm)
    _, cuts, _, _ = best
    shards = []
    for q in range(_NCORES):
        p0, p1 = cuts[q], cuts[q + 1]
        ub, us, mb, ms = [], [], [], []
        for b in range(B):
            lo_b, hi_b = max(p0, b * S), min(p1, (b + 1) * S)
            if lo_b >= hi_b:
                continue
            s0, s1 = lo_b - b * S, hi_b - b * S
            h0, h1 = max(0, s0 - R), min(S, s1 + R)
            up = np.nonzero(unm[b, h0:h1] > 0)[0] + h0
            mp = np.nonzero(valid[b, s0:s1])[0] + s0
            ub.extend([b] * len(up))
            us.extend(up.tolist())
            mb.extend([b] * len(mp))
            ms.extend(mp.tolist())
        shards.append((np.array(ub, np.int64), np.array(us, np.int64),
                       np.array(mb, np.int64), np.array(ms, np.int64)))
    return shards


def kernel(h_L, masked, W_route, b_route, W1, b1, W2, b2, range_r):
    R = int(range_r)
    h_L = np.asarray(h_L, dtype=np.float32)
    masked = np.asarray(masked).astype(bool)
    B, S, D = h_L.shape
    K = W_route.shape[1]
    DC = D // 128
    FM = _F // 128
    KF = K * _F // 128

    unm = (~masked).astype(np.float64)
    cs = np.concatenate([np.zeros((B, 1)), np.cumsum(unm, axis=1)], axis=1)
    idx = np.arange(S)
    hi = np.clip(idx + R, 0, S - 1) + 1
    lo = np.clip(idx - R, 0, S)
    cnt = cs[:, hi] - cs[:, lo] - unm
    valid = masked & (cnt > 0)

    shards = _balance_shards(unm.astype(np.int64), valid, R)

    NUA = _ceil_mult(max(len(us) for _, us, _, _ in shards), 4)
    NU = _ceil_mult(NUA, 128)
    SMA = _ceil_mult(max(len(ms) for _, _, _, ms in shards), 4)
    SM = _ceil_mult(SMA, 128)
    assert NUA <= 512 and SMA <= 512
    JC = NU // 128

    # routing softmax on the host (f32, from the masked tokens' own h)
    logits = h_L.reshape(-1, D) @ np.asarray(W_route, np.float32)
    logits += np.asarray(b_route, np.float32)[None, :]
    logits -= logits.max(axis=1, keepdims=True)
    wexp = np.exp(logits)
    wsm = (wexp / wexp.sum(axis=1, keepdims=True)).reshape(B, S, K)

    # shared weight arrays, pre-laid-out partition-major for linear DMA
    # w1: [K, 128, FM, DC, 128] (fm-major per expert)
    w1b = np.ascontiguousarray(
        W1.astype(BF16)
        .reshape(K, DC, 128, FM, 128)
        .transpose(0, 2, 3, 1, 4)
    )
    w2b = np.ascontiguousarray(
        np.asarray(W2)
        .reshape(KF, 128, DC, 128)
        .transpose(2, 1, 0, 3)
        .astype(BF16)
    )  # [DC, 128, KF, 128]
    b1b = np.ascontiguousarray(
        b1.astype(BF16).reshape(K, _F // 128, 128).transpose(2, 0, 1)
    )  # [128, K, FM]
    b2f = np.asarray(b2, np.float32)

    in_maps = []
    for ub, us, mb, ms in shards:
        nu, sm = len(us), len(ms)
        hgT = np.zeros((D, NUA), dtype=BF16)
        hgT[:, :nu] = h_L[ub, us, :].T.astype(BF16)
        wsel = np.zeros((NU, SMA), dtype=BF16)
        if nu and sm:
            wsel[:nu, :sm] = (
                (np.abs(us[:, None] - ms[None, :]) <= R)
                & (ub[:, None] == mb[None, :])
            ).astype(BF16)
        wi = np.zeros((1, K, SMA), dtype=BF16)
        if sm:
            wi[0, :, :sm] = (wsm[mb, ms, :] / cnt[mb, ms, None]).T.astype(BF16)
        in_maps.append(
            {
                "hgT": np.ascontiguousarray(
                    hgT.reshape(DC, 128, NUA).transpose(1, 0, 2)
                ),
                "wsel": np.ascontiguousarray(
                    wsel.reshape(JC, 128, SMA).transpose(1, 0, 2)
                ),
                "wi": wi,
                "w1": w1b,
                "w2": w2b,
                "b1": b1b,
            }
        )

    key = (NU, SM, SMA, NUA)
    if key not in _GRAPH_CACHE:
        _GRAPH_CACHE[key] = _build_graph(NU, SM, SMA, NUA)
    nc = _GRAPH_CACHE[key]

    from concourse.bass_utils import run_bass_kernel_spmd

    res = run_bass_kernel_spmd(nc, in_maps, core_ids=list(range(_NCORES)))

    out = np.zeros((B, S, D), dtype=np.float32)
    for core, (ub, us, mb, ms) in enumerate(shards):
        if len(ms):
            mixT = res.results[core]["out"].reshape(D, SMA)[:, : len(ms)]
            mixed = mixT.T.astype(np.float32)  # [sm, D]
            mixed += wsm[mb, ms, :] @ b2f  # b2 term, host-side
            mu = mixed.mean(axis=1, keepdims=True)
            var = ((mixed - mu) ** 2).mean(axis=1, keepdims=True)
            out[mb, ms, :] = (mixed - mu) / np.sqrt(var + 1e-5)
    return out

